# revision 1
# baseline (speedup 1.0000x reference)
"""Trainium2 Bass kernel for nn_GCLSTM (B=512, T=32, H=300, 10 neighbors).

Sharding: T is split across 8 cores (4 timesteps each). The reference's
scan carry (`con`) is a pure function of the per-step input, so every
timestep's cells() output can be computed independently; each core only
additionally computes cells() for its t0-1 block to get `lastcon`.
The flat-reshape softmax scramble mixes the whole batch at fixed t, so
keeping B intact per core makes it core-local.

The t=0 step (core 0 only) uses con0 directly instead of
relu(wp@[con;lastcon]+bp); this is folded into the uniform SPMD program
by giving core 0 identity/zero weights for its tau=0 con1 matmul and a
leaky-relu eviction with per-core alpha (1.0 on core 0 -> passthrough).
"""
import os
import sys

for _p in ("/opt/trn_rl_repo", "/root/.axon_site", "/root/.axon_site/_ro/trn_rl_repo",
           "/root/.axon_site/_ro/pypackages"):
    if os.path.isdir(_p) and _p not in sys.path:
        sys.path.append(_p)

import numpy as np
import ml_dtypes
from contextlib import ExitStack

import concourse.bass as bass
import concourse.tile as tile
from concourse.tile_rust import add_dep_helper
from concourse import bacc, mybir
from concourse import bass_utils
from concourse.bass_interp import get_hw_module

BF16 = mybir.dt.bfloat16
F32 = mybir.dt.float32
NPBF = ml_dtypes.bfloat16
AF = mybir.ActivationFunctionType
ALU = mybir.AluOpType

B, T, H = 512, 32, 300
NCORES = 8
TLOC = T // NCORES            # 4 timesteps per core
R = TLOC * B                  # 2048 rows per core (tau*512 + b)
NK = 10                       # neighbors
HC = [(0, 128), (128, 128), (256, 44)]   # H=300 chunking (offset, size)
NRT = R // 128                # 16 row-tiles of 128
# tau' cell blocks: 5 per core (t0-1 .. t0+3), processed in pairs
TPAIRS = [(0, 1), (2, 3), (4,)]


def _scalars_key(sc):
    return tuple(float(v) for v in sc)


_BUILD_CACHE = {}


def _build(f2_8, f2a, b2s, c0):
    nc = bacc.Bacc("TRN2", target_bir_lowering=False, debug=False,
                   enable_asserts=False, num_devices=NCORES)

    def din(name, shape, dt):
        return nc.dram_tensor(name, shape, dt, kind="ExternalInput").ap()

    # --- per-core data inputs ---
    xT = din("xT", [NK, 12, 5 * B], BF16)        # [k, i(11)+ones, tau'*512+b]
    xrow = din("xrow", [R, 190], F32)            # fl8(10) fl10(10) xfus(170)
    exT = din("exT", [5, R], BF16)               # extras features + ones row
    # --- weights (replicated; wpT0/bp0/alpha differ on core 0) ---
    wihT = din("wihT", [NK, 12, 900], BF16)      # per-k lstm weights + bias row
    wtT = din("wtT", [5, 900], BF16)             # target-cell weights + bias row
    wpTn = din("wpTn", [600, 300], BF16)
    wpT0 = din("wpT0", [600, 300], BF16)
    bpn = din("bpn", [128, 3], F32)              # bp per oc chunk (partition p)
    bp0C = din("bp0C", [128, 3], F32)            # core0: +C, else bp
    subC = din("subC", [128, 1], F32)            # core0: C, else 0
    f1A = din("f1A", [300, 201], BF16)           # [v1 | F1top]
    f1B = din("f1B", [301, 201], BF16)           # [0 | F1bot] + b1 ones-row
    f2full = din("f2full", [128, 200], F32)
    w3full = din("w3full", [128, 170], F32)
    afull = din("afull", [128, 10], F32)
    # --- outputs / scratch ---
    preds = nc.dram_tensor("preds", [NRT, 128], F32, kind="ExternalOutput").ap()
    smflat = [nc.dram_tensor(f"smflat{t}", [B * NK], F32, kind="Internal").ap()
              for t in range(TLOC)]
    # write view: [p, i, k] -> flat k*512 + i*128 + p  (one DMA per tau)
    smW = [s.rearrange("(k i p) -> p i k", k=NK, i=4, p=128) for s in smflat]
    # read view: [p, i, j] -> flat (i*128+p)*10 + j  (one DMA per tau)
    smR = [s.rearrange("(i p j) -> p i j", i=4, p=128, j=NK) for s in smflat]
    predsW = preds.rearrange("(t i) p -> t p i", t=TLOC, i=4)

    with tile.TileContext(nc) as tc:
        with ExitStack() as ctx:
            wpool = ctx.enter_context(tc.tile_pool(name="wpool", bufs=1))
            conp = ctx.enter_context(tc.tile_pool(name="conp", bufs=3))
            con1p = ctx.enter_context(tc.tile_pool(name="con1p", bufs=1))
            htarp = ctx.enter_context(tc.tile_pool(name="htarp", bufs=1))
            scrp = ctx.enter_context(tc.tile_pool(name="scrp", bufs=2))
            rowp = ctx.enter_context(tc.tile_pool(name="rowp", bufs=1))
            smallp = ctx.enter_context(tc.tile_pool(name="smallp", bufs=2))
            xkp = ctx.enter_context(tc.tile_pool(name="xkp", bufs=3))
            xrp = ctx.enter_context(tc.tile_pool(name="xrp", bufs=2))
            t0p = ctx.enter_context(tc.tile_pool(name="t0p", bufs=1))
            psA = ctx.enter_context(tc.tile_pool(name="psA", bufs=1, space="PSUM"))
            psB = ctx.enter_context(tc.tile_pool(name="psB", bufs=1, space="PSUM"))
            psC = ctx.enter_context(tc.tile_pool(name="psC", bufs=1, space="PSUM"))

            # ---------------- phase W: load weights/constants ----------------
            def wtile(name, shape, dt, src):
                t = wpool.tile(shape, dt, tag=name)
                nc.sync.dma_start(t[:], src)
                return t

            wt_sb = wtile("wtT", [5, 900], BF16, wtT[:])
            ex_sb = wtile("exT", [5, R], BF16, exT[:])
            f1B_pre = None
            wih_sb = [wtile(f"wih{k}", [12, 900], BF16, xTsrc)
                      for k, xTsrc in ((k, wihT[k]) for k in range(NK))]
            # wpT chunk tiles: 6 h-chunks (con 3 + lastcon 3) x full 300 cols
            wpn_sb, wp0_sb = [], []
            for lbl, src, dstlist in (("n", wpTn, wpn_sb), ("0", wpT0, wp0_sb)):
                for j in range(6):
                    off = (0 if j < 3 else 300) + HC[j % 3][0]
                    sz = HC[j % 3][1]
                    t = wtile(f"wp{lbl}_{j}", [sz, 300], BF16, src[off:off + sz, :])
                    dstlist.append(t)
            HSZ2 = [128, 128, 45]
            f1A_sb = [wtile(f"f1A{c}", [HC[c][1], 201], BF16,
                            f1A[HC[c][0]:HC[c][0] + HC[c][1], :]) for c in range(3)]
            f1B_sb = [wtile(f"f1B{c}", [HSZ2[c], 201], BF16,
                            f1B[HC[c][0]:HC[c][0] + HSZ2[c], :]) for c in range(3)]
            f2_sb = wtile("f2full", [128, 200], F32, f2full[:])
            w3_sb = wtile("w3full", [128, 170], F32, w3full[:])
            a_sb = wtile("afull", [128, 10], F32, afull[:])
            bpn_sb = wtile("bpn", [128, 3], F32, bpn[:])
            bp0_sb = wtile("bp0C", [128, 3], F32, bp0C[:])
            sc_sb = wtile("subC", [128, 1], F32, subC[:])

            GOFF = {"i": 0, "g": 300, "o": 600}
            last_act = [None]
            # static PSUM tiles: pool.tile() per group costs ~1us of
            # TileRelease semaphore latency on the PE; fixed tiles rely on
            # direct producer/consumer deps only.
            psA_t = [psA.tile([128, 1024], F32, name=f"psAs{i}", tag=f"psAs{i}")
                     for i in range(2)]
            psB_t = [psB.tile([128, 512], F32, name=f"psBs{i}", tag=f"psBs{i}")
                     for i in range(2)]
            psC_t = [psC.tile([128, 512], F32, name=f"psCs{i}", tag=f"psCs{i}")
                     for i in range(2)]
            rrA = [0]
            rrB = [0]
            rrC = [0]

            def nextps(tiles, rr):
                t = tiles[rr[0] % len(tiles)]
                rr[0] += 1
                return t


            # LSTM-cell gate pipeline: 3 matmul-evictions + product chain.
            # psum W cols wide (1 or 2 chunks of 512); rhs_fn(j) gives the
            # [K,512] moving operand for sub-chunk j; out_fn(j) the dest AP.
            def gate_chain(c, W, nsub, lhsT_fn, rhs_fn, out_fn, filler=None):
                coff, csz = HC[c]
                tiles = {}
                for g, func in (("i", AF.Sigmoid), ("g", AF.Tanh), ("o", AF.Sigmoid)):
                    if g == "o" and filler is not None:
                        filler()
                    ps = nextps(psA_t, rrA)[0:csz, 0:W]
                    for j in range(nsub):
                        nc.tensor.matmul(ps[:, j * 512:(j + 1) * 512],
                                         lhsT_fn(GOFF[g] + coff, csz),
                                         rhs_fn(j), start=True, stop=True)
                    t = scrp.tile([csz, W], BF16, name=f"sc_{g}", tag=f"sc_{g}")
                    nc.scalar.activation(t[:], ps[:], func)
                    tiles[g] = t
                m1 = scrp.tile([csz, W], BF16, name="sc_m1", tag="sc_m1")
                nc.gpsimd.tensor_mul(m1[:], tiles["i"][:], tiles["g"][:])
                m2 = scrp.tile([csz, W], BF16, name="sc_m2", tag="sc_m2")
                last_act[0] = nc.scalar.activation(m2[:], m1[:], AF.Tanh).ins
                for j in range(nsub):
                    cols = slice(j * 512, (j + 1) * 512)
                    nc.gpsimd.tensor_mul(out_fn(j), tiles["o"][:, cols],
                                         m2[:, cols])

            # ---------------- phase H: target cell (htar) ----------------
            HSZ = [128, 128, 45]   # c2 carries a ones row for the b1 bias
            htar = [htarp.tile([HSZ[c], R], BF16, name=f"htar{c}", tag=f"htar{c}") for c in range(3)]
            nc.sync.dma_start(htar[2][44:45, :], exT[4:5, :])
            for ccp in ((0, 1), (2, 3)):
                for c in range(3):
                    gate_chain(
                        c, 512 * len(ccp), len(ccp),
                        lambda off, sz: wt_sb[:, off:off + sz],
                        lambda j: ex_sb[:, (ccp[0] + j) * 512:(ccp[0] + j + 1) * 512],
                        lambda j, c=c, ccp=ccp: htar[c][0:HC[c][1],
                                                        (ccp[0] + j) * 512:
                                                        (ccp[0] + j + 1) * 512])

            con_tiles = {}   # (taup, k, c) -> [csz, 512] tile

            def emit_cells_chain(pi, k, fillers=None):
                tp = TPAIRS[pi]
                W = 512 * len(tp)
                xks = []
                for taup in tp:
                    xk1 = xkp.tile([12, 512], BF16, name="xk", tag="xk")
                    nc.sync.dma_start(
                        xk1[:], xT[k, :, taup * 512:(taup + 1) * 512])
                    xks.append(xk1)
                for c in range(3):
                    for j, taup in enumerate(tp):
                        con_tiles[(taup, k, c)] = conp.tile(
                            [HC[c][1], 512], BF16,
                            name=f"con_{k}_{c}", tag=f"con_{k}_{c}")
                    gate_chain(
                        c, W, len(tp),
                        lambda off, sz, k=k: wih_sb[k][:, off:off + sz],
                        lambda j, xks=xks: xks[j][:],
                        lambda j, tp=tp, k=k, c=c: con_tiles[(tp[j], k, c)][:],
                        filler=(fillers[c] if fillers else None))

            def con_slice(k, c, taup):
                return con_tiles[(taup, k, c)][:]

            srow = [rowp.tile([128, NK], F32, name=f"srow{rt}", tag=f"srow{rt}") for rt in range(NRT)]
            wdyn = [rowp.tile([128, NK], F32, name=f"wdyn{rt}", tag=f"wdyn{rt}") for rt in range(NRT)]
            wfin = [rowp.tile([128, NK], F32, name=f"wfin{rt}", tag=f"wfin{rt}") for rt in range(NRT)]
            fusc = [rowp.tile([128, 1], F32, name=f"fusc{rt}", tag=f"fusc{rt}") for rt in range(NRT)]

            con1_tiles = {}  # (tau, k, oc) -> tile

            def emit_con1_group(tau, k, oc):
                # con1(tau,k,oc): relu(wp @ [con_t; con_{t-1}] + bp)
                wsel = wp0_sb if tau == 0 else wpn_sb
                if True:
                    ooff, osz = HC[oc]
                    ps = nextps(psB_t + psC_t, rrB)[0:osz, 0:512]
                    for j in range(6):
                        taup = tau + 1 if j < 3 else tau
                        nc.tensor.matmul(
                            ps[:], wsel[j][:, ooff:ooff + osz],
                            con_slice(k, j % 3, taup),
                            start=(j == 0), stop=(j == 5))
                    ct = con1p.tile([osz, 512], BF16, name=f"con1_{k}_{oc}", tag=f"con1_{k}_{oc}")
                    if tau == 0:
                        # relu(psum + bp + C) - C: C=2 on core0 (identity
                        # weights feed con0 here, |con0|<1) -> exact con0
                        tmp0 = t0p.tile([osz, 512], F32, name="c1t0", tag="c1t0")
                        nc.vector.tensor_scalar(
                            tmp0[:], ps[:], bp0_sb[0:osz, oc:oc + 1], 0.0,
                            ALU.add, ALU.max)
                        nc.vector.tensor_scalar(
                            ct[:], tmp0[:], sc_sb[0:osz, 0:1], None,
                            ALU.subtract)
                    elif tau == 2:
                        # ACT is idle in this stretch; DVE is draining fc1(1)
                        nc.scalar.activation(ct[:], ps[:], AF.Relu,
                                             bias=bpn_sb[0:osz, oc:oc + 1])
                    else:
                        nc.vector.tensor_scalar(
                            ct[:], ps[:], bpn_sb[0:osz, oc:oc + 1], 0.0,
                            ALU.add, ALU.max)
                    con1_tiles[(tau, k, oc)] = ct

            def emit_con1_k(tau, k):
                for oc in range(3):
                    emit_con1_group(tau, k, oc)

            def emit_fc1(tau):
                # fc1: Y = htar@F1bot (+b1 ones-row) once per row-tile, then
                # per-k psum = con1-part; col 0 collects s_k = v1 . con1_k.
                # Two neighbors share each psum tile.
                for rt4 in range(4):
                    rt = tau * 4 + rt4
                    cols = (rt * 128, rt * 128 + 128)
                    psY = nextps(psC_t, rrC)[:, 0:201]
                    for c in range(3):
                        nc.tensor.matmul(psY[:], htar[c][:, cols[0]:cols[1]],
                                         f1B_sb[c][:], start=(c == 0), stop=(c == 2))
                    ysb = smallp.tile([128, 201], F32, name="ysb", tag="ysb")
                    nc.vector.tensor_copy(ysb[:], psY[:])
                    ysb2 = ysb[:].unsqueeze(1).broadcast_to([128, 2, 201])
                    for k0 in range(0, NK, 2):
                        psF = nextps(psC_t, rrC)[:, 0:402]
                        for kk in (0, 1):
                            sl = slice(kk * 201, kk * 201 + 201)
                            for oc in range(3):
                                nc.tensor.matmul(
                                    psF[:, sl],
                                    con1_tiles[(tau, k0 + kk, oc)][:, rt4 * 128:rt4 * 128 + 128],
                                    f1A_sb[oc][:], start=(oc == 0), stop=(oc == 2))
                        f1 = smallp.tile([128, 402], F32, name="f1", tag="f1")
                        nc.vector.tensor_tensor(out=f1[:].rearrange("p (u q) -> p u q", u=2),
                                                in0=psF[:].rearrange("p (u q) -> p u q", u=2),
                                                in1=ysb2, op=ALU.add)
                        for kk in (0, 1):
                            nc.vector.tensor_copy(srow[rt][:, k0 + kk:k0 + kk + 1],
                                                  f1[:, kk * 201:kk * 201 + 1])
                            stsc = smallp.tile([128, 200], F32, name="stsc", tag="stsc")
                            nc.vector.scalar_tensor_tensor(
                                stsc[:], f1[:, kk * 201 + 1:kk * 201 + 201], 0.0,
                                f2_sb[:], ALU.max, ALU.mult,
                                accum_out=wdyn[rt][:, k0 + kk:k0 + kk + 1])

            def emit_X(tau):
                # extras: angle/fl8 wdyn terms + relu -> wfin; fusion term
                for rt4 in range(4):
                    rt = tau * 4 + rt4
                    xr = xrp.tile([128, 190], F32, name="xr", tag="xr")
                    nc.sync.dma_start(xr[:], xrow[rt * 128:rt * 128 + 128, :])
                    t1 = smallp.tile([128, 10], F32, name="t1", tag="t1")
                    nc.vector.tensor_sub(t1[:], xr[:, 10:20], a_sb[:])
                    tn = smallp.tile([128, 10], F32, name="tn", tag="tn")
                    nc.vector.tensor_scalar_mul(tn[:], t1[:], -1.0)
                    t2 = smallp.tile([128, 10], F32, name="t2", tag="t2")
                    nc.vector.tensor_max(t2[:], t1[:], tn[:])
                    o1 = smallp.tile([128, 10], F32, name="o1", tag="o1")
                    nc.vector.tensor_scalar_mul(o1[:], xr[:, 0:10], f2_8)
                    wext = smallp.tile([128, 10], F32, name="wext", tag="wext")
                    nc.vector.scalar_tensor_tensor(wext[:], t2[:], f2a, o1[:],
                                                   ALU.mult, ALU.add)
                    wsum = smallp.tile([128, 10], F32, name="wsum", tag="wsum")
                    nc.vector.tensor_add(wsum[:], wdyn[rt][:], wext[:])
                    nc.vector.tensor_scalar(wfin[rt][:], wsum[:], b2s, 0.0,
                                            ALU.add, ALU.max)
                    fsc = smallp.tile([128, 170], F32, name="fsc", tag="fsc")
                    nc.vector.scalar_tensor_tensor(
                        fsc[:], xr[:, 20:190], 0.0, w3_sb[:],
                        ALU.add, ALU.mult, accum_out=fusc[rt][:])

            # ---------------- main pipeline ----------------

            # cells chains interleaved with con1 groups at lag 2, so the PE
            # never head-of-line blocks on a chain's ACT/GP/DVE pipeline.
            def stretch(pi, tau):
                for s in range(NK + 2):
                    fills = None
                    if s >= 2:
                        kf = s - 2
                        fills = [
                            (lambda kf=kf: emit_con1_group(tau, kf, 0)),
                            (lambda kf=kf: emit_con1_group(tau, kf, 1)),
                            (lambda kf=kf: emit_con1_group(tau, kf, 2)),
                        ]
                    if s < NK:
                        emit_cells_chain(pi, s, fillers=fills)
                    elif fills:
                        for f in fills:
                            f()

            def emit_S(tau):
                smtau = smallp.tile([128, 4 * NK], F32, name="smtau", tag="smtau")
                for i4 in range(4):
                    rt = tau * 4 + i4
                    e = smallp.tile([128, NK], F32, name="e", tag="e")
                    nc.scalar.activation(e[:], wfin[rt][:], AF.Exp)
                    z = smallp.tile([128, 1], F32, name="z", tag="z")
                    nc.vector.tensor_reduce(z[:], e[:], mybir.AxisListType.X, ALU.add)
                    rz = smallp.tile([128, 1], F32, name="rz", tag="rz")
                    nc.vector.reciprocal(rz[:], z[:])
                    nc.vector.tensor_scalar_mul(smtau[:, i4 * NK:(i4 + 1) * NK],
                                                e[:], rz[:, 0:1])
                for i4 in range(4):
                    nc.sync.dma_start(smW[tau][:, i4],
                                        smtau[:, i4 * NK:(i4 + 1) * NK])
                wa3t = smallp.tile([128, 4 * NK], F32, name="wa3t", tag="wa3t")
                for i4 in range(4):
                    nc.scalar.dma_start(wa3t[:, i4 * NK:(i4 + 1) * NK],
                                        smR[tau][:, i4])
                prt = smallp.tile([128, 4], F32, name="prt", tag="prt")
                for i4 in range(4):
                    rt = tau * 4 + i4
                    csc = smallp.tile([128, NK], F32, name="csc", tag="csc")
                    cat = smallp.tile([128, 1], F32, name="cat", tag="cat")
                    nc.vector.scalar_tensor_tensor(
                        csc[:], srow[rt][:], 0.0,
                        wa3t[:, i4 * NK:(i4 + 1) * NK],
                        ALU.add, ALU.mult, accum_out=cat[:])
                    nc.vector.scalar_tensor_tensor(prt[:, i4:i4 + 1], cat[:], c0,
                                                   fusc[rt][:], ALU.add, ALU.add)
                nc.scalar.dma_start(predsW[tau], prt[:])

            stretch(0, 0)
            emit_fc1(0)
            emit_X(0)
            emit_S(0)
            stretch(1, 1)
            emit_fc1(1)
            emit_X(1)
            emit_S(1)
            for k in range(NK):
                emit_con1_k(2, k)
            emit_fc1(2)
            emit_X(2)
            emit_S(2)
            stretch(2, 3)
            emit_fc1(3)
            emit_X(3)
            emit_S(3)

    nc.compile()
    nc.m = get_hw_module(nc.m)
    return nc


def _get_nc(f2_8, f2a, b2s, c0):
    key = _scalars_key((f2_8, f2a, b2s, c0))
    if key not in _BUILD_CACHE:
        _BUILD_CACHE[key] = _build(f2_8, f2a, b2s, c0)
    return _BUILD_CACHE[key]


def _softmax(x):
    e = np.exp(x - x.max())
    return e / e.sum()


def prepare_inputs(local_inputs, labels, extras, DisM, AngleM,
                   Wih, b_ih, b_hh, Wt, bt_ih, bt_hh,
                   wp, bp, F1, b1, F2, b2, ff, bff,
                   fuse1, biasf, Wout, biasout, a):
    """Host-side sharding + layout prep. Returns (in_maps, scalars)."""
    f = np.asarray
    local_inputs, labels, extras = f(local_inputs), f(labels), f(extras)
    aa = float(f(a)[0])
    wA = _softmax(f(DisM).astype(np.float64)).astype(np.float32)

    kept = np.r_[0:300, 600:900, 900:1200]      # gates i, g, o (f is dead)
    b_cells = (f(b_ih) + f(b_hh))[:, kept]      # [10, 900]
    bt = (f(bt_ih) + f(bt_hh))[kept]            # [900]

    # wihT[k]: [12, 900] = [Wih[k,kept,:].T ; bias row]
    wihT = np.zeros((NK, 12, 900), np.float32)
    for k in range(NK):
        wihT[k, :11] = f(Wih)[k][kept, :].T
        wihT[k, 11] = b_cells[k]
    wtT = np.zeros((5, 900), np.float32)
    wtT[:4] = f(Wt)[kept, :].T
    wtT[4] = bt

    wpT = f(wp).T.copy()                        # [600, 300]
    wpT0_core0 = np.zeros((600, 300), np.float32)
    wpT0_core0[:300] = np.eye(300, dtype=np.float32)

    v1 = aa * (f(fuse1) @ f(Wout))[:, 0]        # [300]
    f1A = np.zeros((300, 201), np.float32)
    f1A[:, 0] = v1
    f1A[:, 1:] = f(F1)[:300]
    f1B = np.zeros((301, 201), np.float32)
    f1B[:300, 1:] = f(F1)[300:]
    f1B[300, 1:] = f(b1)          # ones-row bias (htar c2 row 44)
    f2full = np.broadcast_to(f(F2)[:200, 0][None, :], (128, 200)).copy()
    afull = np.broadcast_to(f(AngleM)[None, :], (128, 10)).copy()

    ffW = (f(Wout)[:, 0] @ f(ff))               # [17]
    W3 = (1.0 - aa) * np.outer(ffW, wA)         # [17, 10]
    w3full = np.broadcast_to(W3.reshape(-1)[None, :], (128, 170)).copy()

    f2_8 = float(f(F2)[200, 0])
    f2a = float(f(F2)[201, 0]) / 360.0
    b2s = float(f(b2)[0])
    c0 = ((1.0 - aa) * float(f(Wout)[:, 0] @ f(bff)[:, 0])
          + aa * float(f(biasf) @ f(Wout)[:, 0])
          + float(f(biasout)[0]))

    bpcol = np.zeros((128, 3), np.float32)
    for oc, (ooff, osz) in enumerate(HC):
        bpcol[:osz, oc] = f(bp)[ooff:ooff + osz, 0]

    in_maps = []
    for cix in range(NCORES):
        t0 = cix * TLOC
        # x block for cells: t0-1 .. t0+3 (zeros for t=-1 on core 0)
        xblk = np.zeros((5, B, 28, NK), np.float32)
        lo = t0 - 1
        for jj in range(5):
            t = lo + jj
            if 0 <= t < T:
                xblk[jj] = local_inputs[:, t]
        xT = np.zeros((NK, 12, 5 * B), np.float32)
        # xT[k, i, taup*512+b] = xblk[taup, b, i, k]
        xT[:, :11, :] = xblk[:, :, :11, :].transpose(3, 2, 0, 1).reshape(NK, 11, 5 * B)
        xT[:, 11, :] = 1.0

        xloc = local_inputs[:, t0:t0 + TLOC]            # [B, 4, 28, 10]
        xrow = np.empty((R, 190), np.float32)
        xrow[:, 0:10] = xloc[:, :, 8, :].transpose(1, 0, 2).reshape(R, NK)
        xrow[:, 10:20] = xloc[:, :, 10, :].transpose(1, 0, 2).reshape(R, NK)
        xrow[:, 20:190] = xloc[:, :, 11:, :].transpose(1, 0, 2, 3).reshape(R, 170)

        exT = np.ones((5, R), np.float32)
        exT[:4] = f(extras)[:, t0:t0 + TLOC, :4, 0].transpose(2, 1, 0).reshape(4, R)

        core0 = cix == 0
        in_maps.append({
            "xT": xT.astype(NPBF),
            "xrow": xrow,
            "exT": exT.astype(NPBF),
            "wihT": wihT.astype(NPBF),
            "wtT": wtT.astype(NPBF),
            "wpTn": wpT.astype(NPBF),
            "wpT0": (wpT0_core0 if core0 else wpT).astype(NPBF),
            "bpn": bpcol,
            "bp0C": (np.full((128, 3), 2.0, np.float32) if core0 else bpcol),
            "subC": np.full((128, 1), 2.0 if core0 else 0.0, np.float32),
            "f1A": f1A.astype(NPBF),
            "f1B": f1B.astype(NPBF),
            "f2full": f2full,
            "w3full": w3full,
            "afull": afull,
        })
    return in_maps, (f2_8, f2a, b2s, c0)


def kernel(local_inputs, labels, extras, DisM, AngleM,
           Wih, b_ih, b_hh, Wt, bt_ih, bt_hh,
           wp, bp, F1, b1, F2, b2, ff, bff,
           fuse1, biasf, Wout, biasout, a, _trace=False, _tmpdir=None):
    in_maps, (f2_8, f2a, b2s, c0) = prepare_inputs(
        local_inputs, labels, extras, DisM, AngleM,
        Wih, b_ih, b_hh, Wt, bt_ih, bt_hh, wp, bp, F1, b1, F2, b2,
        ff, bff, fuse1, biasf, Wout, biasout, a)
    nc = _get_nc(f2_8, f2a, b2s, c0)
    res = bass_utils.run_bass_kernel_spmd(
        nc, in_maps, core_ids=list(range(NCORES)), trace=_trace, tmpdir=_tmpdir)

    preds = np.empty((T, B, 1), np.float32)
    for cix in range(NCORES):
        out = res.results[cix]["preds"].reshape(TLOC, B)
        preds[cix * TLOC:(cix + 1) * TLOC, :, 0] = out

    labels_r = np.ascontiguousarray(
        np.transpose(np.asarray(labels), (1, 0, 2, 3)).reshape(T, B, 1))
    kernel._last_result = res
    return preds, labels_r

